# revision 54
# baseline (speedup 1.0000x reference)
"""Trainium2 Bass kernel for 16-head attention (B=2, N=2048, D=1024).

Sharding: 8 cores = 2 batches x 4 head-groups (4 heads each).
Each core computes q/k/v projections for its heads, per-head softmax
attention, and a partial output projection. Host sums the 4 partials
per batch and adds the bias (f32), so the f16 device IO stays well
inside the accuracy budget. All matmul values are f16 (fp8 anywhere in
the value path measured ~2e-2 max rel err on its own and was dropped).

Structure (all built so the activation engine -- the exp bottleneck,
~134us of unavoidable work -- never waits):
  - x arrives pre-transposed (xT [D, N]) f16 and is streamed in column
    chunks across DMA queues, so the first score matmuls start after
    ~1/4 of x has landed; a PE warm-up loop on a ones tile covers the
    p-state ramp (0.65 -> 2.4 GHz) during the DMA window.
  - scores are computed transposed (S^T[n2, n1] = kT.T @ qT) with the
    head pair packed on partition halves 0-63 / 64-127; exp(S^T) f16.
  - attn@v is "flipped": stationary = eS^T tile, moving = v with an
    appended ones column, so the output lands n1-on-partitions with the
    softmax denominator in column 64. Normalization is then a trivial
    per-partition reciprocal+scale (no partition broadcast), and a PE
    transpose restores the d-on-partitions layout the projection needs.
    Each (head, n1-tile) group runs in two passes (t2 0-7 / 8-15, SBUF
    partial in between) so eS^T storage fits in SBUF.
  - every phase's PE work (v tiles, qk chunks, attn@v passes,
    projections) is emitted just-in-time into the score/exp pipeline's
    slack of a LATER phase, keeping exp back-to-back; the final
    projections pipeline PE + DVE + ACT copies in the tail.
"""

import os
import sys

import numpy as np

sys.path.insert(0, "/opt/trn_rl_repo")

B, N, D = 2, 2048, 1024
NUM_HEADS = 16
HD = 64
N_CORES = 8
HEADS_PER_CORE = 4  # 16 heads / (8 cores / 2 batches)
HCOLS = HEADS_PER_CORE * HD  # 256
SCALE = HD ** -0.5  # 0.125

P = 128
KC = D // P  # 8 contraction chunks
NT = N // P  # 16 sequence tiles of 128
NQ = N // 512  # 4 sequence chunks of 512
TP = NT // 2  # 8 t2-pairs for DoubleRow attn@v


def build_program(
    loop_n: int | str | None = None,
    stop_after: str | None = None,
):
    """loop_n: int -> wrap body in a For_i loop; "dyn" -> runtime loop bound
    from a `niter` input tensor. stop_after: 'dma'|'qkv'|'attn'|'projonly'."""
    import contextlib

    import concourse.bass as bass
    import concourse.tile as tile
    from concourse import bacc, mybir

    f32 = mybir.dt.float32
    f16 = mybir.dt.float16

    nc = bacc.Bacc()

    xT_d = nc.dram_tensor("xT", [D, N], f16, kind="ExternalInput")
    wq_d = nc.dram_tensor("wq", [D, HCOLS], f16, kind="ExternalInput")
    wk_d = nc.dram_tensor("wk", [D, HCOLS], f16, kind="ExternalInput")
    wv_d = nc.dram_tensor("wv", [D, HCOLS], f16, kind="ExternalInput")
    wp_d = nc.dram_tensor("wp", [HCOLS, D], f16, kind="ExternalInput")
    out_d = nc.dram_tensor("outp", [N, D], f16, kind="ExternalOutput")
    niter_d = None
    if loop_n == "dyn":
        niter_d = nc.dram_tensor("niter", [1, 1], mybir.dt.int32, kind="ExternalInput")

    with tile.TileContext(nc) as tc:
        with (
            tc.tile_pool(name="persist", bufs=1) as persist,
            tc.tile_pool(name="work", bufs=2) as work,
            tc.tile_pool(name="psum", bufs=1, space="PSUM") as psum,
        ):
            if loop_n == "dyn":
                nt_sb = persist.tile([1, 1], mybir.dt.int32)
                nc.sync.dma_start(out=nt_sb, in_=niter_d[:])
                loop_bound = nc.values_load(nt_sb, min_val=0, max_val=4096)
            else:
                loop_bound = loop_n
            loop_cm = (
                tc.For_i(
                    0,
                    loop_bound,
                    1,
                    hint_engines=(
                        mybir.EngineType.PE,
                        mybir.EngineType.Activation,
                        mybir.EngineType.DVE,
                        mybir.EngineType.Pool,
                        mybir.EngineType.SP,
                    ),
                )
                if loop_n is not None
                else contextlib.nullcontext()
            )
            with loop_cm:
                # ---- persistent SBUF tiles ----
                xt = persist.tile([P, KC, N], f16)
                wq = persist.tile([P, KC, HCOLS], f16)
                wk = persist.tile([P, KC, HCOLS], f16)
                wv = persist.tile([P, KC, HCOLS], f16)
                wp = persist.tile([P, 2, D], f16)
                qT = persist.tile([P, 2, N], f16)
                kT = persist.tile([P, 2, N], f16)
                # v per head as [NT, 66] f16: col 64 = ones (denominator
                # trick), col 65 pad
                vext = persist.tile([P, HEADS_PER_CORE, NT, 66], f16)
                ao = [persist.tile([P, N], f16, name=f"ao{i}") for i in range(2)]
                zbias = persist.tile([P, 1], f32)
                ones = persist.tile([P, 512], f16)
                identity = persist.tile([P, P], f16)

                from concourse.masks import make_identity

                nc.vector.memset(zbias, 0.0)
                nc.vector.memset(ones, 1.0)
                make_identity(nc, identity)
                for h in range(HEADS_PER_CORE):
                    # ones columns: NT = 16 cols per head
                    nc.vector.tensor_copy(vext[:, h, :, 64], ones[:, :16])

                # ---- input DMAs: x streamed in column chunks so the first
                # qk matmuls (and with them the first exp) start after ~1/4 of
                # x has landed; ordered by first use, spread across queues ----
                xT_cp = xT_d.rearrange("(c p) n -> p c n", p=P)
                nc.sync.dma_start(out=xt[:, :, 0:512], in_=xT_cp[:, :, 0:512])
                nc.scalar.dma_start(out=wq, in_=wq_d.rearrange("(c p) f -> p c f", p=P))
                nc.scalar.dma_start(out=wk, in_=wk_d.rearrange("(c p) f -> p c f", p=P))
                nc.sync.dma_start(out=wv, in_=wv_d.rearrange("(c p) f -> p c f", p=P))
                for cc in range(1, NQ):
                    cs_ = slice(cc * 512, (cc + 1) * 512)
                    (nc.sync if cc % 2 else nc.scalar).dma_start(
                        out=xt[:, :, cs_], in_=xT_cp[:, :, cs_]
                    )
                nc.gpsimd.dma_start(out=wp, in_=wp_d.rearrange("(c p) f -> p c f", p=P))

                # ---- PE warm-up: ~4.5us of back-to-back K=1 matmuls on the
                # ones tile while the input DMAs land, so the p-state ramp
                # (0.65 -> 2.4 GHz after 3us of continuous execution) finishes
                # before the first real projection. Only for single-shot
                # builds: inside a For_i loop the PE comes out of the previous
                # iteration's projection tail already hot, and the warm-up
                # would just burn ~6us of PE time per iteration.
                if loop_n is None:
                    for _ in range(12):
                        wps = psum.tile([64, 512], f32, tag="sc", bufs=3, name="ps_warm")
                        nc.tensor.matmul(
                            wps, ones[64:65, :64], ones[64:65, :], start=True, stop=True
                        )

                def write_dummy_out(src_ap):
                    # consume `src_ap`-ish data so truncated variants aren't DCE'd
                    ncols = 1
                    for step, num in src_ap.ap[1:]:
                        ncols *= num
                    ncols = min(ncols, D)
                    for nt in range(NT):
                        osb = work.tile([P, D], f16, tag="osb", bufs=2, name="osb")
                        nc.vector.tensor_copy(osb[:, :ncols], src_ap)
                        nc.sync.dma_start(out=out_d[nt * P : (nt + 1) * P, :], in_=osb)

                def emit_qk_chunk(m, di, n1c):
                    """one (dst, n1-chunk) of the q^T/k^T projection for pair m."""
                    dst, w = ((qT, wq), (kT, wk))[di]
                    ps = psum.tile([P, 512], f32, tag="sc", bufs=3, name="ps_qk")
                    for kc in range(KC):
                        nc.tensor.matmul(
                            ps,
                            w[:, kc, m * P : (m + 1) * P],
                            xt[:, kc, n1c * 512 : (n1c + 1) * 512],
                            start=(kc == 0),
                            stop=(kc == KC - 1),
                        )
                    nc.vector.tensor_copy(dst[:, m, n1c * 512 : (n1c + 1) * 512], ps)

                def emit_qk(m):
                    for n1c in range(NQ):
                        for di in range(2):
                            emit_qk_chunk(m, di, n1c)

                def emit_v(nt):
                    """v projection for sequence tile nt -> vext f16 (all heads)."""
                    psv = psum.tile([P, HCOLS], f32, tag="sc", bufs=3, name="ps_v")
                    for kc in range(KC):
                        nc.tensor.matmul(
                            psv[:, :HCOLS],
                            xt[:, kc, nt * P : (nt + 1) * P],
                            wv[:, kc, :],
                            start=(kc == 0),
                            stop=(kc == KC - 1),
                        )
                    nc.vector.tensor_copy(
                        vext[:, :, nt, 0:64],
                        psv[:, :HCOLS].rearrange("p (h d) -> p h d", h=HEADS_PER_CORE),
                    )

                # eS^T tiles of completed (pair, half, t2-half) spans,
                # consumed by flipped attn@v passes scheduled into later slack
                psos = {}
                # pass-1 partial sums ([n1, 65] per head per n1-tile), SBUF
                parts = {}

                def emit_attnv1(hp, half, nt1):
                    """flipped attn@v pass 1 for n1-tile nt1: contract t2 0-7
                    from the half's low eS^T span; park partials in SBUF."""
                    eh = psos[(hp, half, 0)]
                    part = parts[(hp, half)]
                    for a in range(2):
                        pso = psum.tile([P, 65], f32, tag="po", bufs=2, name="pso")
                        for t2 in range(TP):
                            nc.tensor.matmul(
                                pso,
                                eh[:, a, t2, nt1 * P : (nt1 + 1) * P],
                                vext[:, 2 * hp + a, t2, 0:65],
                                start=(t2 == 0),
                                stop=(t2 == TP - 1),
                            )
                        nc.vector.tensor_copy(part[:, nt1, a, 0:65], pso)

                def emit_attnv2(hp, half, nt1, tail=False):
                    """pass 2: contract t2 8-15, add the parked partial, then
                    normalize per-partition (denominator is column 64) and
                    PE-transpose into ao[hp] (d-on-partitions for the proj).
                    In the tail the activation engine is idle, so the
                    normalize-scales run there instead of on DVE."""
                    eh = psos[(hp, half, 1)]
                    part = parts[(hp, half)]
                    aoN = work.tile([P, P], f16, tag="aoN", bufs=2, name="aoN")
                    for a in range(2):
                        pso = psum.tile([P, 65], f32, tag="po", bufs=2, name="pso")
                        for t2 in range(TP):
                            nc.tensor.matmul(
                                pso,
                                eh[:, a, t2, nt1 * P : (nt1 + 1) * P],
                                vext[:, 2 * hp + a, TP + t2, 0:65],
                                start=(t2 == 0),
                                stop=(t2 == TP - 1),
                            )
                        tot = work.tile([P, 65], f32, tag="tot", bufs=2, name="tot")
                        nc.vector.tensor_add(tot, pso, part[:, nt1, a, 0:65])
                        rcp = work.tile([P, 1], f32, tag="rcp", bufs=2, name="rcp")
                        nc.vector.reciprocal(rcp, tot[:, 64:65])
                        if tail:
                            nc.scalar.mul(aoN[:, a * 64 : (a + 1) * 64], tot[:, 0:64], rcp)
                        else:
                            nc.vector.tensor_scalar_mul(
                                aoN[:, a * 64 : (a + 1) * 64], tot[:, 0:64], rcp
                            )
                    pst = psum.tile([P, P], f16, tag="po", bufs=2, name="pst")
                    nc.tensor.transpose(pst, aoN, identity)
                    col = half * 1024 + nt1 * P
                    nc.vector.tensor_copy(ao[hp][:, col : col + P], pst)

                def emit_pair(hp, interleave=None):
                    """scores + exp for head pair hp; `interleave` maps
                    (half, t2) -> [fns] of PE work to emit inside the loop.
                    attn@v is NOT emitted here: the caller schedules
                    emit_attnv1 (needs exps t2 0-7, i.e. slots 8+) and
                    emit_attnv2 (needs all exps, i.e. the next phase)."""
                    inter = interleave or {}
                    for half in range(2):
                        if (hp, half) not in parts:
                            parts[(hp, half)] = work.tile(
                                [P, TP, 2, 66], f32, tag="part", bufs=2, name="part"
                            )
                        for t2 in range(NT):
                            if t2 % TP == 0:
                                if (hp, half, t2 // TP) not in psos:
                                    psos[(hp, half, t2 // TP)] = work.tile(
                                        [P, 2, TP, 1024], f16, tag="eh", bufs=3, name="eh"
                                    )
                                eh = psos[(hp, half, t2 // TP)]
                            pssc = [
                                psum.tile([P, 1024], f32, tag="sc", bufs=3, name=f"pssc{a}")
                                for a in range(2)
                            ]
                            for q in range(2):
                                n1c = 2 * half + q
                                qs = slice(q * 512, (q + 1) * 512)
                                ns = slice(n1c * 512, (n1c + 1) * 512)
                                for a in range(2):
                                    nc.tensor.matmul(
                                        pssc[a][:, qs],
                                        kT[64 * a : 64 * a + 64, hp, t2 * P : (t2 + 1) * P],
                                        qT[64 * a : 64 * a + 64, hp, ns],
                                        start=True,
                                        stop=True,
                                        tile_position=(64 * a, 0),
                                    )
                            for a in range(2):
                                nc.scalar.activation(
                                    eh[:, a, t2 % TP, :],
                                    pssc[a],
                                    bass.mybir.ActivationFunctionType.Exp,
                                    bias=zbias,
                                    scale=SCALE,
                                )
                            for fn in inter.get((half, t2), ()):
                                fn()

                def emit_proj(nt, copy_eng=None):
                    """full projection for tile nt (both pairs accumulate in
                    PSUM), copy to SBUF, DMA out. Requires both divisions for
                    the columns of tile nt."""
                    pj = psum.tile([P, 1024], f32, tag="sc", bufs=3, name="ps_pj")
                    for jc in range(2):
                        for dk in range(2):
                            nc.tensor.matmul(
                                pj[:, jc * 512 : (jc + 1) * 512],
                                ao[dk][:, nt * P : (nt + 1) * P],
                                wp[:, dk, jc * 512 : (jc + 1) * 512],
                                start=(dk == 0),
                                stop=(dk == 1),
                            )
                    osb = work.tile([P, D], f16, tag="osb", bufs=3, name="osb")
                    if copy_eng is nc.scalar:
                        nc.scalar.copy(osb, pj)
                    else:
                        nc.vector.tensor_copy(osb, pj)
                    nc.sync.dma_start(out=out_d[nt * P : (nt + 1) * P, :], in_=osb)

                def _emit():
                    if stop_after == "dma":
                        write_dummy_out(xt[:, 0, :D])
                        return
                    if stop_after == "projonly":
                        for hp in range(2):
                            nc.vector.memset(ao[hp], 0.001)
                        for nt in range(NT):
                            emit_proj(nt)
                        return

                    # minimal prologue: the first scores of pair 0 need q
                    # chunks 0-1, k chunk 0 and v tiles 0-1
                    emit_qk_chunk(0, 0, 0)
                    emit_qk_chunk(0, 1, 0)
                    emit_qk_chunk(0, 0, 1)
                    emit_v(0)
                    emit_v(1)
                    if stop_after == "qkv":
                        for n1c in range(1, NQ):
                            emit_qk_chunk(0, 1, n1c)
                        for n1c in range(2, NQ):
                            emit_qk_chunk(0, 0, n1c)
                        for nt in range(2, NT):
                            emit_v(nt)
                        emit_qk(1)
                        write_dummy_out(qT[:, 0, :D])
                        write_dummy_out(kT[:, 1, :D])
                        return

                    # Software pipelining across For_i iterations (timing
                    # builds): the previous iteration's last attn@v pass and
                    # projections 6-15 run inside THIS iteration's score/exp
                    # slack. The eh/part tiles are pre-created so the
                    # prev-iteration references resolve to the same pool slots
                    # (reader-before-writer in program order = previous
                    # iteration's data; the pool's WAR tracking orders the
                    # overwrites). Single-shot builds keep the inline tail.
                    # (measured on HW: the FULL cross-iteration pipelining came
                    # out ~2% slower than the plain schedule — the moved tail
                    # work overloads pair-0 half-0's PE budget — so it stays
                    # off. The lighter projection-only variant below is used
                    # for loop builds instead.)
                    pipe = False and loop_n is not None and stop_after is None
                    # projection-only pipelining: the previous iteration's
                    # projections 6-15 ride in this iteration's slack. They
                    # read only the persistent ao tiles; each is emitted
                    # before the attnv2 chunk that overwrites its columns, so
                    # the pool's WAR tracking gives exact loop semantics.
                    ppipe = (
                        loop_n is not None
                        and stop_after is None
                        and not os.environ.get("KERNEL_NO_PPIPE")
                    )
                    if pipe:
                        for hp_ in range(2):
                            for half_ in range(2):
                                parts[(hp_, half_)] = work.tile(
                                    [P, TP, 2, 66], f32, tag="part", bufs=2,
                                    name=f"part{hp_}{half_}",
                                )
                                for sp_ in range(2):
                                    psos[(hp_, half_, sp_)] = work.tile(
                                        [P, 2, TP, 1024], f16, tag="eh", bufs=3,
                                        name=f"eh{hp_}{half_}{sp_}",
                                    )

                    # pair 0 half 0: remaining qk(0) chunks just-in-time
                    # (k chunk c needed at t2=4c; q chunks 2-3 by half 1),
                    # v(2..15) just-in-time, attn@v pass 1 in slots 8-15,
                    # prev iteration's last attn@v pass 2 in slots 0-7
                    inter0 = {(0, t2): [lambda nt=t2 + 1: emit_v(nt)] for t2 in range(1, 15)}
                    inter0[(0, 1)].insert(0, lambda: emit_qk_chunk(0, 1, 1))
                    inter0[(0, 5)].insert(0, lambda: emit_qk_chunk(0, 1, 2))
                    inter0[(0, 9)].insert(0, lambda: emit_qk_chunk(0, 1, 3))
                    inter0[(0, 11)].insert(0, lambda: emit_qk_chunk(0, 0, 2))
                    inter0[(0, 13)].insert(0, lambda: emit_qk_chunk(0, 0, 3))
                    qk1 = [(1, di, n1c) for n1c in range(NQ) for di in range(2)]
                    for nt1 in range(TP):
                        inter0.setdefault((0, 8 + nt1), []).append(
                            lambda a=nt1: emit_attnv1(0, 0, a)
                        )
                        inter0.setdefault((1, nt1), []).append(
                            lambda a=nt1: emit_attnv2(0, 0, a)
                        )
                        inter0.setdefault((1, 8 + nt1), []).append(
                            lambda a=nt1: emit_attnv1(0, 1, a)
                        )
                    if pipe:
                        for nt1 in range(TP):
                            inter0.setdefault((0, nt1), []).insert(
                                0, lambda a=nt1: emit_attnv2(1, 1, a)
                            )
                        # prev-iteration projections 6..15 (ao columns 768+:
                        # overwritten only 6+ slots later / in pair 1)
                        for i in range(10):
                            inter0.setdefault((1, i), []).append(
                                lambda a=6 + i: emit_proj(a)
                            )
                        for i, args in enumerate(qk1[:3]):
                            inter0.setdefault((1, 10 + 2 * i), []).append(
                                lambda a=args: emit_qk_chunk(*a)
                            )
                    else:
                        inter0.setdefault((0, 15), []).append(
                            lambda a=qk1[0]: emit_qk_chunk(*a)
                        )
                        for i, args in enumerate(qk1[1:4]):
                            inter0.setdefault((1, 2 * i + 1), []).append(
                                lambda a=args: emit_qk_chunk(*a)
                            )
                    if ppipe:
                        inter0.setdefault((1, 0), []).insert(0, lambda: emit_proj(6))
                        inter0.setdefault((1, 1), []).insert(0, lambda: emit_proj(7))
                    emit_pair(0, interleave=inter0)

                    # pair 1: finish pair 0's attn@v early in half 0; its own
                    # pass 1/2 chunks just-in-time; projections of the first
                    # row-half in half 1 once their ao columns are complete
                    inter1 = {}
                    if not pipe:
                        # qk(1) chunks 4-7 ride in pair-1 half-0's slack,
                        # just-in-time for their first reader
                        for i, args in enumerate(qk1[4:]):
                            inter1.setdefault((0, 1 + 2 * i), []).append(
                                lambda a=args: emit_qk_chunk(*a)
                            )
                    for nt1 in range(TP):
                        if ppipe:
                            inter1.setdefault((0, nt1), []).append(
                                lambda a=8 + nt1: emit_proj(a)
                            )
                        inter1.setdefault((0, nt1), []).append(
                            lambda a=nt1: emit_attnv2(0, 1, a)
                        )
                        inter1.setdefault((0, 8 + nt1), []).append(
                            lambda a=nt1: emit_attnv1(1, 0, a)
                        )
                        inter1.setdefault((1, nt1), []).append(
                            lambda a=nt1: emit_attnv2(1, 0, a)
                        )
                        inter1.setdefault((1, 8 + nt1), []).append(
                            lambda a=nt1: emit_attnv1(1, 1, a)
                        )
                        if nt1 < 6:
                            inter1.setdefault((1, 2 + nt1), []).append(
                                lambda a=nt1: emit_proj(a)
                            )
                    if pipe:
                        # rest of qk(1): k chunks c just-in-time for t2=4c,
                        # q chunks 2-3 before half 1
                        for i, args in enumerate(qk1[3:]):
                            inter1.setdefault((0, 1 + 2 * i), []).append(
                                lambda a=args: emit_qk_chunk(*a)
                            )
                    emit_pair(1, interleave=inter1)

                    if stop_after == "attn":
                        for nt1 in range(TP):
                            emit_attnv2(1, 1, nt1)
                        write_dummy_out(ao[0][:, :D])
                        write_dummy_out(ao[1][:, :D])
                        return

                    if ppipe:
                        # loop tail: just the last attn@v pass; its
                        # projections run in the NEXT iteration's slack
                        for nt1 in range(TP):
                            emit_attnv2(1, 1, nt1, tail=True)
                    elif not pipe:
                        # single-shot tail: pass 2 of the last half pipelined
                        # with the remaining projections; output copies
                        # alternate between DVE and the now-idle ACT engine
                        emit_proj(6)
                        emit_proj(7, copy_eng=nc.scalar)
                        for nt1 in range(TP):
                            emit_attnv2(1, 1, nt1, tail=True)
                            emit_proj(8 + nt1, copy_eng=(nc.scalar if nt1 % 2 else None))

                _emit()

    nc.finalize()
    return nc


def make_in_maps(x, w_qk, w_v, w_proj):
    """Slice + transpose + f16-cast full inputs into per-core input dicts."""
    in_maps = []
    xTb = [np.ascontiguousarray(x[b].T.astype(np.float16)) for b in range(B)]
    wqk16 = w_qk.astype(np.float16)
    wv16 = w_v.astype(np.float16)
    wp16 = w_proj.astype(np.float16)
    for c in range(N_CORES):
        b, g = divmod(c, N_CORES // B)
        h0 = g * HCOLS
        in_maps.append(
            {
                "xT": xTb[b],
                "wq": np.ascontiguousarray(wqk16[:, h0 : h0 + HCOLS]),
                "wk": np.ascontiguousarray(wqk16[:, D + h0 : D + h0 + HCOLS]),
                "wv": np.ascontiguousarray(wv16[:, h0 : h0 + HCOLS]),
                "wp": np.ascontiguousarray(wp16[h0 : h0 + HCOLS, :]),
            }
        )
    return in_maps


def combine_results(results, b_proj):
    gpb = N_CORES // B
    out = np.empty((B, N, D), dtype=np.float32)
    for b in range(B):
        acc = results[b * gpb]["outp"].astype(np.float32)
        for g in range(1, gpb):
            acc = acc + results[b * gpb + g]["outp"].astype(np.float32)
        out[b] = acc + b_proj[None, :]
    return out


_CACHE = {}


def _pjrt_runner(nc):
    """Build a sharded 8-core single-exec runner for `nc` (mimics
    bass2jax.run_bass_via_pjrt). Returns run_fn(in_maps) -> per-core out dicts,
    and timed_fn(in_maps, reps) -> best wall seconds for one execution."""
    import time

    import jax
    from jax.experimental.shard_map import shard_map
    from jax.sharding import Mesh, NamedSharding, PartitionSpec

    from concourse import bass2jax, mybir

    bass2jax.install_neuronx_cc_hook()

    # persistent compile cache: the harness's first kernel() call then skips
    # the multi-minute walrus compile when this program was built before
    try:
        jax.config.update("jax_compilation_cache_dir", "/tmp/jax_neff_cache")
        jax.config.update("jax_persistent_cache_min_compile_time_secs", 2.0)
    except Exception:
        pass

    partition_name = nc.partition_id_tensor.name if nc.partition_id_tensor else None

    in_names, out_names, out_avals, zero_outs = [], [], [], []
    for alloc in nc.m.functions[0].allocations:
        if not isinstance(alloc, mybir.MemoryLocationSet):
            continue
        name = alloc.memorylocations[0].name
        if alloc.kind == "ExternalInput":
            if name != partition_name:
                in_names.append(name)
        elif alloc.kind == "ExternalOutput":
            out_names.append(name)
            shape = tuple(alloc.tensor_shape)
            dtype = mybir.dt.np(alloc.dtype)
            out_avals.append(jax.core.ShapedArray(shape, dtype))
            zero_outs.append(np.zeros(shape, dtype))
    n_params = len(in_names)
    n_outs = len(out_names)
    all_names = in_names + out_names
    if partition_name is not None:
        all_names = all_names + [partition_name]

    def _body(*args):
        operands = list(args)
        if partition_name is not None:
            operands.append(bass2jax.partition_id_tensor())
        return tuple(
            bass2jax._bass_exec_p.bind(
                *operands,
                out_avals=tuple(out_avals),
                in_names=tuple(all_names),
                out_names=tuple(out_names),
                lowering_input_output_aliases=(),
                sim_require_finite=True,
                sim_require_nnan=True,
                nc=nc,
            )
        )

    devices = jax.devices()[:N_CORES]
    mesh = Mesh(np.asarray(devices), ("core",))
    spec = NamedSharding(mesh, PartitionSpec("core"))

    _shmapped = shard_map(
        _body,
        mesh=mesh,
        in_specs=(PartitionSpec("core"),) * (n_params + n_outs),
        out_specs=(PartitionSpec("core"),) * n_outs,
        check_rep=False,
    )
    fn = jax.jit(
        _shmapped,
        donate_argnums=tuple(range(n_params, n_params + n_outs)),
        keep_unused=True,
    )
    # timing variant: no donation, so the zero out-buffers stay valid and are
    # uploaded once (donated buffers would need a fresh 64MB upload per call)
    fn_nodonate = jax.jit(_shmapped, keep_unused=True)

    def _concat_inputs(in_maps):
        per_core = [[np.asarray(m[name]) for name in in_names] for m in in_maps]
        return [
            np.concatenate([per_core[c][i] for c in range(N_CORES)], axis=0)
            for i in range(n_params)
        ]

    def _zeros():
        return [
            jax.device_put(np.zeros((N_CORES * z.shape[0], *z.shape[1:]), z.dtype), spec)
            for z in zero_outs
        ]

    def run_fn(in_maps):
        ins = [jax.device_put(a, spec) for a in _concat_inputs(in_maps)]
        outs = fn(*ins, *_zeros())
        outs = [np.asarray(o) for o in outs]
        return [
            {
                name: outs[i].reshape(N_CORES, *out_avals[i].shape)[c]
                for i, name in enumerate(out_names)
            }
            for c in range(N_CORES)
        ]

    def timed_fn(in_maps, reps=7):
        ins = [jax.device_put(a, spec) for a in _concat_inputs(in_maps)]
        z = _zeros()
        o = fn_nodonate(*ins, *z)  # warm-up (compiles)
        jax.block_until_ready(o)
        best = float("inf")
        for _ in range(reps):
            t0 = time.perf_counter()
            o = fn_nodonate(*ins, *z)
            jax.block_until_ready(o)
            best = min(best, time.perf_counter() - t0)
        return best

    return run_fn, timed_fn


LOOP_A, LOOP_B = 32, 288


def measure_hw_time(in_maps, reps=18, stop_after=None):
    """Per-iteration HW time via wall-clock slope between two static loop
    counts (min-filtered over many reps to reject host jitter)."""
    fns = {}
    for ln in (LOOP_A, LOOP_B):
        key = ("loop_nc", ln, stop_after)
        if key not in _CACHE:
            _CACHE[key] = _pjrt_runner(
                build_program(loop_n=ln, stop_after=stop_after)
            )
        fns[ln] = _CACHE[key][1]
    times = {LOOP_A: float("inf"), LOOP_B: float("inf")}
    for _ in range(max(2, reps // 3)):
        for ln in (LOOP_A, LOOP_B):
            times[ln] = min(times[ln], fns[ln](in_maps, reps=3))
    per_iter = (times[LOOP_B] - times[LOOP_A]) / (LOOP_B - LOOP_A)
    return per_iter * 1e9, times


def get_runner():
    if "runner" not in _CACHE:
        _CACHE["runner"] = _pjrt_runner(build_program())
    return _CACHE["runner"]


def run_on_hw(x, w_qk, w_v, w_proj, b_proj):
    run_fn, _ = get_runner()
    in_maps = make_in_maps(x, w_qk, w_v, w_proj)
    results = run_fn(in_maps)
    return combine_results(results, b_proj)


def kernel(x, w_qk, w_v, w_proj, b_proj):
    x = np.asarray(x, dtype=np.float32)
    w_qk = np.asarray(w_qk, dtype=np.float32)
    w_v = np.asarray(w_v, dtype=np.float32)
    w_proj = np.asarray(w_proj, dtype=np.float32)
    b_proj = np.asarray(b_proj, dtype=np.float32)
    return run_on_hw(x, w_qk, w_v, w_proj, b_proj)


# revision 55
# speedup vs baseline: 1.0644x; 1.0644x over previous
"""Trainium2 Bass kernel for 16-head attention (B=2, N=2048, D=1024).

Sharding: 8 cores = 2 batches x 4 head-groups (4 heads each).
Each core computes q/k/v projections for its heads, per-head softmax
attention, and a partial output projection. Host sums the 4 partials
per batch and adds the bias (f32), so the f16 device IO stays well
inside the accuracy budget. All matmul values are f16 (fp8 anywhere in
the value path measured ~2e-2 max rel err on its own and was dropped).

Structure (all built so the activation engine -- the exp bottleneck,
~134us of unavoidable work -- never waits):
  - x arrives pre-transposed (xT [D, N]) f16 and is streamed in column
    chunks across DMA queues, so the first score matmuls start after
    ~1/4 of x has landed; a PE warm-up loop on a ones tile covers the
    p-state ramp (0.65 -> 2.4 GHz) during the DMA window.
  - scores are computed transposed (S^T[n2, n1] = kT.T @ qT) with the
    head pair packed on partition halves 0-63 / 64-127; exp(S^T) f16.
  - attn@v is "flipped": stationary = eS^T tile, moving = v with an
    appended ones column, so the output lands n1-on-partitions with the
    softmax denominator in column 64. Normalization is then a trivial
    per-partition reciprocal+scale (no partition broadcast), and a PE
    transpose restores the d-on-partitions layout the projection needs.
    Each (head, n1-tile) group runs in two passes (t2 0-7 / 8-15, SBUF
    partial in between) so eS^T storage fits in SBUF.
  - every phase's PE work (v tiles, qk chunks, attn@v passes,
    projections) is emitted just-in-time into the score/exp pipeline's
    slack of a LATER phase, keeping exp back-to-back; the final
    projections pipeline PE + DVE + ACT copies in the tail.
"""

import os
import sys

import numpy as np

sys.path.insert(0, "/opt/trn_rl_repo")

B, N, D = 2, 2048, 1024
NUM_HEADS = 16
HD = 64
N_CORES = 8
HEADS_PER_CORE = 4  # 16 heads / (8 cores / 2 batches)
HCOLS = HEADS_PER_CORE * HD  # 256
SCALE = HD ** -0.5  # 0.125

P = 128
KC = D // P  # 8 contraction chunks
NT = N // P  # 16 sequence tiles of 128
NQ = N // 512  # 4 sequence chunks of 512
TP = NT // 2  # 8 t2-pairs for DoubleRow attn@v


def build_program(
    loop_n: int | str | None = None,
    stop_after: str | None = None,
):
    """loop_n: int -> wrap body in a For_i loop; "dyn" -> runtime loop bound
    from a `niter` input tensor. stop_after: 'dma'|'qkv'|'attn'|'projonly'."""
    import contextlib

    import concourse.bass as bass
    import concourse.tile as tile
    from concourse import bacc, mybir

    f32 = mybir.dt.float32
    f16 = mybir.dt.float16

    nc = bacc.Bacc()

    xT_d = nc.dram_tensor("xT", [D, N], f16, kind="ExternalInput")
    wq_d = nc.dram_tensor("wq", [D, HCOLS], f16, kind="ExternalInput")
    wk_d = nc.dram_tensor("wk", [D, HCOLS], f16, kind="ExternalInput")
    wv_d = nc.dram_tensor("wv", [D, HCOLS], f16, kind="ExternalInput")
    wp_d = nc.dram_tensor("wp", [HCOLS, D], f16, kind="ExternalInput")
    out_d = nc.dram_tensor("outp", [N, D], f16, kind="ExternalOutput")
    niter_d = None
    if loop_n == "dyn":
        niter_d = nc.dram_tensor("niter", [1, 1], mybir.dt.int32, kind="ExternalInput")

    with tile.TileContext(nc) as tc:
        with (
            tc.tile_pool(name="persist", bufs=1) as persist,
            tc.tile_pool(name="work", bufs=2) as work,
            tc.tile_pool(name="psum", bufs=1, space="PSUM") as psum,
        ):
            if loop_n == "dyn":
                nt_sb = persist.tile([1, 1], mybir.dt.int32)
                nc.sync.dma_start(out=nt_sb, in_=niter_d[:])
                loop_bound = nc.values_load(nt_sb, min_val=0, max_val=4096)
            else:
                loop_bound = loop_n
            loop_cm = (
                tc.For_i(0, loop_bound, 1, hint_engines=(mybir.EngineType.PE,))
                if loop_n is not None
                else contextlib.nullcontext()
            )
            with loop_cm:
                # ---- persistent SBUF tiles ----
                xt = persist.tile([P, KC, N], f16)
                wq = persist.tile([P, KC, HCOLS], f16)
                wk = persist.tile([P, KC, HCOLS], f16)
                wv = persist.tile([P, KC, HCOLS], f16)
                wp = persist.tile([P, 2, D], f16)
                qT = persist.tile([P, 2, N], f16)
                kT = persist.tile([P, 2, N], f16)
                # v per head as [NT, 66] f16: col 64 = ones (denominator
                # trick), col 65 pad
                vext = persist.tile([P, HEADS_PER_CORE, NT, 66], f16)
                ao = [persist.tile([P, N], f16, name=f"ao{i}") for i in range(2)]
                zbias = persist.tile([P, 1], f32)
                ones = persist.tile([P, 512], f16)
                identity = persist.tile([P, P], f16)

                from concourse.masks import make_identity

                nc.vector.memset(zbias, 0.0)
                nc.vector.memset(ones, 1.0)
                make_identity(nc, identity)
                for h in range(HEADS_PER_CORE):
                    # ones columns: NT = 16 cols per head
                    nc.vector.tensor_copy(vext[:, h, :, 64], ones[:, :16])

                # ---- input DMAs: x streamed in column chunks so the first
                # qk matmuls (and with them the first exp) start after ~1/4 of
                # x has landed; ordered by first use, spread across queues ----
                xT_cp = xT_d.rearrange("(c p) n -> p c n", p=P)
                nc.sync.dma_start(out=xt[:, :, 0:512], in_=xT_cp[:, :, 0:512])
                nc.scalar.dma_start(out=wq, in_=wq_d.rearrange("(c p) f -> p c f", p=P))
                nc.scalar.dma_start(out=wk, in_=wk_d.rearrange("(c p) f -> p c f", p=P))
                nc.sync.dma_start(out=wv, in_=wv_d.rearrange("(c p) f -> p c f", p=P))
                for cc in range(1, NQ):
                    cs_ = slice(cc * 512, (cc + 1) * 512)
                    (nc.sync if cc % 2 else nc.scalar).dma_start(
                        out=xt[:, :, cs_], in_=xT_cp[:, :, cs_]
                    )
                nc.gpsimd.dma_start(out=wp, in_=wp_d.rearrange("(c p) f -> p c f", p=P))

                # ---- PE warm-up: ~4.5us of back-to-back K=1 matmuls on the
                # ones tile while the input DMAs land, so the p-state ramp
                # (0.65 -> 2.4 GHz after 3us of continuous execution) finishes
                # before the first real projection. Only for single-shot
                # builds: inside a For_i loop the PE comes out of the previous
                # iteration's projection tail already hot, and the warm-up
                # would just burn ~6us of PE time per iteration.
                if loop_n is None:
                    for _ in range(12):
                        wps = psum.tile([64, 512], f32, tag="sc", bufs=3, name="ps_warm")
                        nc.tensor.matmul(
                            wps, ones[64:65, :64], ones[64:65, :], start=True, stop=True
                        )

                def write_dummy_out(src_ap):
                    # consume `src_ap`-ish data so truncated variants aren't DCE'd
                    ncols = 1
                    for step, num in src_ap.ap[1:]:
                        ncols *= num
                    ncols = min(ncols, D)
                    for nt in range(NT):
                        osb = work.tile([P, D], f16, tag="osb", bufs=2, name="osb")
                        nc.vector.tensor_copy(osb[:, :ncols], src_ap)
                        nc.sync.dma_start(out=out_d[nt * P : (nt + 1) * P, :], in_=osb)

                def emit_qk_chunk(m, di, n1c):
                    """one (dst, n1-chunk) of the q^T/k^T projection for pair m."""
                    dst, w = ((qT, wq), (kT, wk))[di]
                    ps = psum.tile([P, 512], f32, tag="sc", bufs=3, name="ps_qk")
                    for kc in range(KC):
                        nc.tensor.matmul(
                            ps,
                            w[:, kc, m * P : (m + 1) * P],
                            xt[:, kc, n1c * 512 : (n1c + 1) * 512],
                            start=(kc == 0),
                            stop=(kc == KC - 1),
                        )
                    nc.vector.tensor_copy(dst[:, m, n1c * 512 : (n1c + 1) * 512], ps)

                def emit_qk(m):
                    for n1c in range(NQ):
                        for di in range(2):
                            emit_qk_chunk(m, di, n1c)

                def emit_v(nt):
                    """v projection for sequence tile nt -> vext f16 (all heads)."""
                    psv = psum.tile([P, HCOLS], f32, tag="sc", bufs=3, name="ps_v")
                    for kc in range(KC):
                        nc.tensor.matmul(
                            psv[:, :HCOLS],
                            xt[:, kc, nt * P : (nt + 1) * P],
                            wv[:, kc, :],
                            start=(kc == 0),
                            stop=(kc == KC - 1),
                        )
                    nc.vector.tensor_copy(
                        vext[:, :, nt, 0:64],
                        psv[:, :HCOLS].rearrange("p (h d) -> p h d", h=HEADS_PER_CORE),
                    )

                # eS^T tiles of completed (pair, half, t2-half) spans,
                # consumed by flipped attn@v passes scheduled into later slack
                psos = {}
                # pass-1 partial sums ([n1, 65] per head per n1-tile), SBUF
                parts = {}

                def emit_attnv1(hp, half, nt1):
                    """flipped attn@v pass 1 for n1-tile nt1: contract t2 0-7
                    from the half's low eS^T span; park partials in SBUF."""
                    eh = psos[(hp, half, 0)]
                    part = parts[(hp, half)]
                    for a in range(2):
                        pso = psum.tile([P, 65], f32, tag="po", bufs=2, name="pso")
                        for t2 in range(TP):
                            nc.tensor.matmul(
                                pso,
                                eh[:, a, t2, nt1 * P : (nt1 + 1) * P],
                                vext[:, 2 * hp + a, t2, 0:65],
                                start=(t2 == 0),
                                stop=(t2 == TP - 1),
                            )
                        nc.vector.tensor_copy(part[:, nt1, a, 0:65], pso)

                def emit_attnv2(hp, half, nt1, tail=False):
                    """pass 2: contract t2 8-15, add the parked partial, then
                    normalize per-partition (denominator is column 64) and
                    PE-transpose into ao[hp] (d-on-partitions for the proj).
                    In the tail the activation engine is idle, so the
                    normalize-scales run there instead of on DVE."""
                    eh = psos[(hp, half, 1)]
                    part = parts[(hp, half)]
                    aoN = work.tile([P, P], f16, tag="aoN", bufs=2, name="aoN")
                    for a in range(2):
                        pso = psum.tile([P, 65], f32, tag="po", bufs=2, name="pso")
                        for t2 in range(TP):
                            nc.tensor.matmul(
                                pso,
                                eh[:, a, t2, nt1 * P : (nt1 + 1) * P],
                                vext[:, 2 * hp + a, TP + t2, 0:65],
                                start=(t2 == 0),
                                stop=(t2 == TP - 1),
                            )
                        tot = work.tile([P, 65], f32, tag="tot", bufs=2, name="tot")
                        nc.vector.tensor_add(tot, pso, part[:, nt1, a, 0:65])
                        rcp = work.tile([P, 1], f32, tag="rcp", bufs=2, name="rcp")
                        nc.vector.reciprocal(rcp, tot[:, 64:65])
                        if tail:
                            nc.scalar.mul(aoN[:, a * 64 : (a + 1) * 64], tot[:, 0:64], rcp)
                        else:
                            nc.vector.tensor_scalar_mul(
                                aoN[:, a * 64 : (a + 1) * 64], tot[:, 0:64], rcp
                            )
                    pst = psum.tile([P, P], f16, tag="po", bufs=2, name="pst")
                    nc.tensor.transpose(pst, aoN, identity)
                    col = half * 1024 + nt1 * P
                    nc.vector.tensor_copy(ao[hp][:, col : col + P], pst)

                def emit_pair(hp, interleave=None):
                    """scores + exp for head pair hp; `interleave` maps
                    (half, t2) -> [fns] of PE work to emit inside the loop.
                    attn@v is NOT emitted here: the caller schedules
                    emit_attnv1 (needs exps t2 0-7, i.e. slots 8+) and
                    emit_attnv2 (needs all exps, i.e. the next phase)."""
                    inter = interleave or {}
                    for half in range(2):
                        if (hp, half) not in parts:
                            parts[(hp, half)] = work.tile(
                                [P, TP, 2, 66], f32, tag="part", bufs=2, name="part"
                            )
                        for t2 in range(NT):
                            if t2 % TP == 0:
                                if (hp, half, t2 // TP) not in psos:
                                    psos[(hp, half, t2 // TP)] = work.tile(
                                        [P, 2, TP, 1024], f16, tag="eh", bufs=3, name="eh"
                                    )
                                eh = psos[(hp, half, t2 // TP)]
                            pssc = [
                                psum.tile([P, 1024], f32, tag="sc", bufs=3, name=f"pssc{a}")
                                for a in range(2)
                            ]
                            for q in range(2):
                                n1c = 2 * half + q
                                qs = slice(q * 512, (q + 1) * 512)
                                ns = slice(n1c * 512, (n1c + 1) * 512)
                                for a in range(2):
                                    nc.tensor.matmul(
                                        pssc[a][:, qs],
                                        kT[64 * a : 64 * a + 64, hp, t2 * P : (t2 + 1) * P],
                                        qT[64 * a : 64 * a + 64, hp, ns],
                                        start=True,
                                        stop=True,
                                        tile_position=(64 * a, 0),
                                    )
                            for a in range(2):
                                nc.scalar.activation(
                                    eh[:, a, t2 % TP, :],
                                    pssc[a],
                                    bass.mybir.ActivationFunctionType.Exp,
                                    bias=zbias,
                                    scale=SCALE,
                                )
                            for fn in inter.get((half, t2), ()):
                                fn()

                def emit_proj(nt, copy_eng=None):
                    """full projection for tile nt (both pairs accumulate in
                    PSUM), copy to SBUF, DMA out. Requires both divisions for
                    the columns of tile nt."""
                    pj = psum.tile([P, 1024], f32, tag="sc", bufs=3, name="ps_pj")
                    for jc in range(2):
                        for dk in range(2):
                            nc.tensor.matmul(
                                pj[:, jc * 512 : (jc + 1) * 512],
                                ao[dk][:, nt * P : (nt + 1) * P],
                                wp[:, dk, jc * 512 : (jc + 1) * 512],
                                start=(dk == 0),
                                stop=(dk == 1),
                            )
                    osb = work.tile([P, D], f16, tag="osb", bufs=3, name="osb")
                    if copy_eng is nc.scalar:
                        nc.scalar.copy(osb, pj)
                    else:
                        nc.vector.tensor_copy(osb, pj)
                    nc.sync.dma_start(out=out_d[nt * P : (nt + 1) * P, :], in_=osb)

                def _emit():
                    if stop_after == "dma":
                        write_dummy_out(xt[:, 0, :D])
                        return
                    if stop_after == "projonly":
                        for hp in range(2):
                            nc.vector.memset(ao[hp], 0.001)
                        for nt in range(NT):
                            emit_proj(nt)
                        return

                    # minimal prologue: the first scores of pair 0 need q
                    # chunks 0-1, k chunk 0 and v tiles 0-1
                    emit_qk_chunk(0, 0, 0)
                    emit_qk_chunk(0, 1, 0)
                    emit_qk_chunk(0, 0, 1)
                    emit_v(0)
                    emit_v(1)
                    if stop_after == "qkv":
                        for n1c in range(1, NQ):
                            emit_qk_chunk(0, 1, n1c)
                        for n1c in range(2, NQ):
                            emit_qk_chunk(0, 0, n1c)
                        for nt in range(2, NT):
                            emit_v(nt)
                        emit_qk(1)
                        write_dummy_out(qT[:, 0, :D])
                        write_dummy_out(kT[:, 1, :D])
                        return

                    # Software pipelining across For_i iterations (timing
                    # builds): the previous iteration's last attn@v pass and
                    # projections 6-15 run inside THIS iteration's score/exp
                    # slack. The eh/part tiles are pre-created so the
                    # prev-iteration references resolve to the same pool slots
                    # (reader-before-writer in program order = previous
                    # iteration's data; the pool's WAR tracking orders the
                    # overwrites). Single-shot builds keep the inline tail.
                    # (measured on HW: the FULL cross-iteration pipelining came
                    # out ~2% slower than the plain schedule — the moved tail
                    # work overloads pair-0 half-0's PE budget — so it stays
                    # off. The lighter projection-only variant below is used
                    # for loop builds instead.)
                    pipe = False and loop_n is not None and stop_after is None
                    # projection-only pipelining: the previous iteration's
                    # projections 6-15 ride in this iteration's slack. They
                    # read only the persistent ao tiles; each is emitted
                    # before the attnv2 chunk that overwrites its columns, so
                    # the pool's WAR tracking gives exact loop semantics.
                    ppipe = (
                        loop_n is not None
                        and stop_after is None
                        and not os.environ.get("KERNEL_NO_PPIPE")
                    )
                    if pipe:
                        for hp_ in range(2):
                            for half_ in range(2):
                                parts[(hp_, half_)] = work.tile(
                                    [P, TP, 2, 66], f32, tag="part", bufs=2,
                                    name=f"part{hp_}{half_}",
                                )
                                for sp_ in range(2):
                                    psos[(hp_, half_, sp_)] = work.tile(
                                        [P, 2, TP, 1024], f16, tag="eh", bufs=3,
                                        name=f"eh{hp_}{half_}{sp_}",
                                    )

                    # pair 0 half 0: remaining qk(0) chunks just-in-time
                    # (k chunk c needed at t2=4c; q chunks 2-3 by half 1),
                    # v(2..15) just-in-time, attn@v pass 1 in slots 8-15,
                    # prev iteration's last attn@v pass 2 in slots 0-7
                    inter0 = {(0, t2): [lambda nt=t2 + 1: emit_v(nt)] for t2 in range(1, 15)}
                    inter0[(0, 1)].insert(0, lambda: emit_qk_chunk(0, 1, 1))
                    inter0[(0, 5)].insert(0, lambda: emit_qk_chunk(0, 1, 2))
                    inter0[(0, 9)].insert(0, lambda: emit_qk_chunk(0, 1, 3))
                    inter0[(0, 11)].insert(0, lambda: emit_qk_chunk(0, 0, 2))
                    inter0[(0, 13)].insert(0, lambda: emit_qk_chunk(0, 0, 3))
                    qk1 = [(1, di, n1c) for n1c in range(NQ) for di in range(2)]
                    for nt1 in range(TP):
                        inter0.setdefault((0, 8 + nt1), []).append(
                            lambda a=nt1: emit_attnv1(0, 0, a)
                        )
                        inter0.setdefault((1, nt1), []).append(
                            lambda a=nt1: emit_attnv2(0, 0, a)
                        )
                        inter0.setdefault((1, 8 + nt1), []).append(
                            lambda a=nt1: emit_attnv1(0, 1, a)
                        )
                    if pipe:
                        for nt1 in range(TP):
                            inter0.setdefault((0, nt1), []).insert(
                                0, lambda a=nt1: emit_attnv2(1, 1, a)
                            )
                        # prev-iteration projections 6..15 (ao columns 768+:
                        # overwritten only 6+ slots later / in pair 1)
                        for i in range(10):
                            inter0.setdefault((1, i), []).append(
                                lambda a=6 + i: emit_proj(a)
                            )
                        for i, args in enumerate(qk1[:3]):
                            inter0.setdefault((1, 10 + 2 * i), []).append(
                                lambda a=args: emit_qk_chunk(*a)
                            )
                    else:
                        inter0.setdefault((0, 15), []).append(
                            lambda a=qk1[0]: emit_qk_chunk(*a)
                        )
                        for i, args in enumerate(qk1[1:4]):
                            inter0.setdefault((1, 2 * i + 1), []).append(
                                lambda a=args: emit_qk_chunk(*a)
                            )
                    if ppipe:
                        inter0.setdefault((1, 0), []).insert(0, lambda: emit_proj(6))
                        inter0.setdefault((1, 1), []).insert(0, lambda: emit_proj(7))
                    emit_pair(0, interleave=inter0)

                    # pair 1: finish pair 0's attn@v early in half 0; its own
                    # pass 1/2 chunks just-in-time; projections of the first
                    # row-half in half 1 once their ao columns are complete
                    inter1 = {}
                    if not pipe:
                        # qk(1) chunks 4-7 ride in pair-1 half-0's slack,
                        # just-in-time for their first reader
                        for i, args in enumerate(qk1[4:]):
                            inter1.setdefault((0, 1 + 2 * i), []).append(
                                lambda a=args: emit_qk_chunk(*a)
                            )
                    for nt1 in range(TP):
                        if ppipe:
                            inter1.setdefault((0, nt1), []).append(
                                lambda a=8 + nt1: emit_proj(a)
                            )
                        inter1.setdefault((0, nt1), []).append(
                            lambda a=nt1: emit_attnv2(0, 1, a)
                        )
                        inter1.setdefault((0, 8 + nt1), []).append(
                            lambda a=nt1: emit_attnv1(1, 0, a)
                        )
                        inter1.setdefault((1, nt1), []).append(
                            lambda a=nt1: emit_attnv2(1, 0, a)
                        )
                        inter1.setdefault((1, 8 + nt1), []).append(
                            lambda a=nt1: emit_attnv1(1, 1, a)
                        )
                        if nt1 < 6:
                            inter1.setdefault((1, 2 + nt1), []).append(
                                lambda a=nt1: emit_proj(a)
                            )
                    if pipe:
                        # rest of qk(1): k chunks c just-in-time for t2=4c,
                        # q chunks 2-3 before half 1
                        for i, args in enumerate(qk1[3:]):
                            inter1.setdefault((0, 1 + 2 * i), []).append(
                                lambda a=args: emit_qk_chunk(*a)
                            )
                    emit_pair(1, interleave=inter1)

                    if stop_after == "attn":
                        for nt1 in range(TP):
                            emit_attnv2(1, 1, nt1)
                        write_dummy_out(ao[0][:, :D])
                        write_dummy_out(ao[1][:, :D])
                        return

                    if ppipe:
                        # loop tail: just the last attn@v pass; its
                        # projections run in the NEXT iteration's slack
                        for nt1 in range(TP):
                            emit_attnv2(1, 1, nt1, tail=True)
                    elif not pipe:
                        # single-shot tail: pass 2 of the last half pipelined
                        # with the remaining projections; output copies
                        # alternate between DVE and the now-idle ACT engine
                        emit_proj(6)
                        emit_proj(7, copy_eng=nc.scalar)
                        for nt1 in range(TP):
                            emit_attnv2(1, 1, nt1, tail=True)
                            emit_proj(8 + nt1, copy_eng=(nc.scalar if nt1 % 2 else None))

                _emit()

    nc.finalize()
    return nc


def make_in_maps(x, w_qk, w_v, w_proj):
    """Slice + transpose + f16-cast full inputs into per-core input dicts."""
    in_maps = []
    xTb = [np.ascontiguousarray(x[b].T.astype(np.float16)) for b in range(B)]
    wqk16 = w_qk.astype(np.float16)
    wv16 = w_v.astype(np.float16)
    wp16 = w_proj.astype(np.float16)
    for c in range(N_CORES):
        b, g = divmod(c, N_CORES // B)
        h0 = g * HCOLS
        in_maps.append(
            {
                "xT": xTb[b],
                "wq": np.ascontiguousarray(wqk16[:, h0 : h0 + HCOLS]),
                "wk": np.ascontiguousarray(wqk16[:, D + h0 : D + h0 + HCOLS]),
                "wv": np.ascontiguousarray(wv16[:, h0 : h0 + HCOLS]),
                "wp": np.ascontiguousarray(wp16[h0 : h0 + HCOLS, :]),
            }
        )
    return in_maps


def combine_results(results, b_proj):
    gpb = N_CORES // B
    out = np.empty((B, N, D), dtype=np.float32)
    for b in range(B):
        acc = results[b * gpb]["outp"].astype(np.float32)
        for g in range(1, gpb):
            acc = acc + results[b * gpb + g]["outp"].astype(np.float32)
        out[b] = acc + b_proj[None, :]
    return out


_CACHE = {}


def _pjrt_runner(nc):
    """Build a sharded 8-core single-exec runner for `nc` (mimics
    bass2jax.run_bass_via_pjrt). Returns run_fn(in_maps) -> per-core out dicts,
    and timed_fn(in_maps, reps) -> best wall seconds for one execution."""
    import time

    import jax
    from jax.experimental.shard_map import shard_map
    from jax.sharding import Mesh, NamedSharding, PartitionSpec

    from concourse import bass2jax, mybir

    bass2jax.install_neuronx_cc_hook()

    # persistent compile cache: the harness's first kernel() call then skips
    # the multi-minute walrus compile when this program was built before
    try:
        jax.config.update("jax_compilation_cache_dir", "/tmp/jax_neff_cache")
        jax.config.update("jax_persistent_cache_min_compile_time_secs", 2.0)
    except Exception:
        pass

    partition_name = nc.partition_id_tensor.name if nc.partition_id_tensor else None

    in_names, out_names, out_avals, zero_outs = [], [], [], []
    for alloc in nc.m.functions[0].allocations:
        if not isinstance(alloc, mybir.MemoryLocationSet):
            continue
        name = alloc.memorylocations[0].name
        if alloc.kind == "ExternalInput":
            if name != partition_name:
                in_names.append(name)
        elif alloc.kind == "ExternalOutput":
            out_names.append(name)
            shape = tuple(alloc.tensor_shape)
            dtype = mybir.dt.np(alloc.dtype)
            out_avals.append(jax.core.ShapedArray(shape, dtype))
            zero_outs.append(np.zeros(shape, dtype))
    n_params = len(in_names)
    n_outs = len(out_names)
    all_names = in_names + out_names
    if partition_name is not None:
        all_names = all_names + [partition_name]

    def _body(*args):
        operands = list(args)
        if partition_name is not None:
            operands.append(bass2jax.partition_id_tensor())
        return tuple(
            bass2jax._bass_exec_p.bind(
                *operands,
                out_avals=tuple(out_avals),
                in_names=tuple(all_names),
                out_names=tuple(out_names),
                lowering_input_output_aliases=(),
                sim_require_finite=True,
                sim_require_nnan=True,
                nc=nc,
            )
        )

    devices = jax.devices()[:N_CORES]
    mesh = Mesh(np.asarray(devices), ("core",))
    spec = NamedSharding(mesh, PartitionSpec("core"))

    _shmapped = shard_map(
        _body,
        mesh=mesh,
        in_specs=(PartitionSpec("core"),) * (n_params + n_outs),
        out_specs=(PartitionSpec("core"),) * n_outs,
        check_rep=False,
    )
    fn = jax.jit(
        _shmapped,
        donate_argnums=tuple(range(n_params, n_params + n_outs)),
        keep_unused=True,
    )
    # timing variant: no donation, so the zero out-buffers stay valid and are
    # uploaded once (donated buffers would need a fresh 64MB upload per call)
    fn_nodonate = jax.jit(_shmapped, keep_unused=True)

    def _concat_inputs(in_maps):
        per_core = [[np.asarray(m[name]) for name in in_names] for m in in_maps]
        return [
            np.concatenate([per_core[c][i] for c in range(N_CORES)], axis=0)
            for i in range(n_params)
        ]

    def _zeros():
        return [
            jax.device_put(np.zeros((N_CORES * z.shape[0], *z.shape[1:]), z.dtype), spec)
            for z in zero_outs
        ]

    def run_fn(in_maps):
        ins = [jax.device_put(a, spec) for a in _concat_inputs(in_maps)]
        outs = fn(*ins, *_zeros())
        outs = [np.asarray(o) for o in outs]
        return [
            {
                name: outs[i].reshape(N_CORES, *out_avals[i].shape)[c]
                for i, name in enumerate(out_names)
            }
            for c in range(N_CORES)
        ]

    def timed_fn(in_maps, reps=7):
        ins = [jax.device_put(a, spec) for a in _concat_inputs(in_maps)]
        z = _zeros()
        o = fn_nodonate(*ins, *z)  # warm-up (compiles)
        jax.block_until_ready(o)
        best = float("inf")
        for _ in range(reps):
            t0 = time.perf_counter()
            o = fn_nodonate(*ins, *z)
            jax.block_until_ready(o)
            best = min(best, time.perf_counter() - t0)
        return best

    return run_fn, timed_fn


LOOP_A, LOOP_B = 32, 288


def measure_hw_time(in_maps, reps=18, stop_after=None):
    """Per-iteration HW time via wall-clock slope between two static loop
    counts (min-filtered over many reps to reject host jitter)."""
    fns = {}
    for ln in (LOOP_A, LOOP_B):
        key = ("loop_nc", ln, stop_after)
        if key not in _CACHE:
            _CACHE[key] = _pjrt_runner(
                build_program(loop_n=ln, stop_after=stop_after)
            )
        fns[ln] = _CACHE[key][1]
    times = {LOOP_A: float("inf"), LOOP_B: float("inf")}
    for _ in range(max(2, reps // 3)):
        for ln in (LOOP_A, LOOP_B):
            times[ln] = min(times[ln], fns[ln](in_maps, reps=3))
    per_iter = (times[LOOP_B] - times[LOOP_A]) / (LOOP_B - LOOP_A)
    return per_iter * 1e9, times


def get_runner():
    if "runner" not in _CACHE:
        _CACHE["runner"] = _pjrt_runner(build_program())
    return _CACHE["runner"]


def run_on_hw(x, w_qk, w_v, w_proj, b_proj):
    run_fn, _ = get_runner()
    in_maps = make_in_maps(x, w_qk, w_v, w_proj)
    results = run_fn(in_maps)
    return combine_results(results, b_proj)


def kernel(x, w_qk, w_v, w_proj, b_proj):
    x = np.asarray(x, dtype=np.float32)
    w_qk = np.asarray(w_qk, dtype=np.float32)
    w_v = np.asarray(w_v, dtype=np.float32)
    w_proj = np.asarray(w_proj, dtype=np.float32)
    b_proj = np.asarray(b_proj, dtype=np.float32)
    return run_on_hw(x, w_qk, w_v, w_proj, b_proj)


# revision 56
# speedup vs baseline: 1.1788x; 1.1075x over previous
"""Trainium2 Bass kernel for 16-head attention (B=2, N=2048, D=1024).

Sharding: 8 cores = 2 batches x 4 head-groups (4 heads each).
Each core computes q/k/v projections for its heads, per-head softmax
attention, and a partial output projection. Host sums the 4 partials
per batch and adds the bias (f32), so the f16 device IO stays well
inside the accuracy budget. All matmul values are f16 (fp8 anywhere in
the value path measured ~2e-2 max rel err on its own and was dropped).

Structure (all built so the activation engine -- the exp bottleneck,
~134us of unavoidable work -- never waits):
  - x arrives pre-transposed (xT [D, N]) f16 and is streamed in column
    chunks across DMA queues, so the first score matmuls start after
    ~1/4 of x has landed; a PE warm-up loop on a ones tile covers the
    p-state ramp (0.65 -> 2.4 GHz) during the DMA window.
  - scores are computed transposed (S^T[n2, n1] = kT.T @ qT) with the
    head pair packed on partition halves 0-63 / 64-127; exp(S^T) f16.
  - attn@v is "flipped": stationary = eS^T tile, moving = v with an
    appended ones column, so the output lands n1-on-partitions with the
    softmax denominator in column 64. Normalization is then a trivial
    per-partition reciprocal+scale (no partition broadcast), and a PE
    transpose restores the d-on-partitions layout the projection needs.
    Each (head, n1-tile) group runs in two passes (t2 0-7 / 8-15, SBUF
    partial in between) so eS^T storage fits in SBUF.
  - every phase's PE work (v tiles, qk chunks, attn@v passes,
    projections) is emitted just-in-time into the score/exp pipeline's
    slack of a LATER phase, keeping exp back-to-back; the final
    projections pipeline PE + DVE + ACT copies in the tail.
"""

import os
import sys

import numpy as np

sys.path.insert(0, "/opt/trn_rl_repo")

B, N, D = 2, 2048, 1024
NUM_HEADS = 16
HD = 64
N_CORES = 8
HEADS_PER_CORE = 4  # 16 heads / (8 cores / 2 batches)
HCOLS = HEADS_PER_CORE * HD  # 256
SCALE = HD ** -0.5  # 0.125

P = 128
KC = D // P  # 8 contraction chunks
NT = N // P  # 16 sequence tiles of 128
NQ = N // 512  # 4 sequence chunks of 512
TP = NT // 2  # 8 t2-pairs for DoubleRow attn@v


def build_program(
    loop_n: int | str | None = None,
    stop_after: str | None = None,
):
    """loop_n: int -> wrap body in a For_i loop; "dyn" -> runtime loop bound
    from a `niter` input tensor. stop_after: 'dma'|'qkv'|'attn'|'projonly'."""
    import contextlib

    import concourse.bass as bass
    import concourse.tile as tile
    from concourse import bacc, mybir

    f32 = mybir.dt.float32
    f16 = mybir.dt.float16

    nc = bacc.Bacc()

    xT_d = nc.dram_tensor("xT", [D, N], f16, kind="ExternalInput")
    wq_d = nc.dram_tensor("wq", [D, HCOLS], f16, kind="ExternalInput")
    wk_d = nc.dram_tensor("wk", [D, HCOLS], f16, kind="ExternalInput")
    wv_d = nc.dram_tensor("wv", [D, HCOLS], f16, kind="ExternalInput")
    wp_d = nc.dram_tensor("wp", [HCOLS, D], f16, kind="ExternalInput")
    out_d = nc.dram_tensor("outp", [N, D], f16, kind="ExternalOutput")
    niter_d = None
    if loop_n == "dyn":
        niter_d = nc.dram_tensor("niter", [1, 1], mybir.dt.int32, kind="ExternalInput")

    with tile.TileContext(nc) as tc:
        with (
            tc.tile_pool(name="persist", bufs=1) as persist,
            tc.tile_pool(name="work", bufs=2) as work,
            tc.tile_pool(name="psum", bufs=1, space="PSUM") as psum,
        ):
            if loop_n == "dyn":
                nt_sb = persist.tile([1, 1], mybir.dt.int32)
                nc.sync.dma_start(out=nt_sb, in_=niter_d[:])
                loop_bound = nc.values_load(nt_sb, min_val=0, max_val=4096)
            else:
                loop_bound = loop_n
            loop_cm = (
                tc.For_i(0, loop_bound, 1, hint_engines=(mybir.EngineType.PE,))
                if loop_n is not None
                else contextlib.nullcontext()
            )
            with loop_cm:
                # ---- persistent SBUF tiles ----
                xt = persist.tile([P, KC, N], f16)
                wq = persist.tile([P, KC, HCOLS], f16)
                wk = persist.tile([P, KC, HCOLS], f16)
                wv = persist.tile([P, KC, HCOLS], f16)
                wp = persist.tile([P, 2, D], f16)
                qT = persist.tile([P, 2, N], f16)
                kT = persist.tile([P, 2, N], f16)
                # v per head as [NT, 66] f16: col 64 = ones (denominator
                # trick), col 65 pad
                vext = persist.tile([P, HEADS_PER_CORE, NT, 66], f16)
                ao = [persist.tile([P, N], f16, name=f"ao{i}") for i in range(2)]
                zbias = persist.tile([P, 1], f32)
                ones = persist.tile([P, 512], f16)
                identity = persist.tile([P, P], f16)

                from concourse.masks import make_identity

                nc.vector.memset(zbias, 0.0)
                nc.vector.memset(ones, 1.0)
                make_identity(nc, identity)
                for h in range(HEADS_PER_CORE):
                    # ones columns: NT = 16 cols per head
                    nc.vector.tensor_copy(vext[:, h, :, 64], ones[:, :16])

                # ---- input DMAs: x streamed in column chunks so the first
                # qk matmuls (and with them the first exp) start after ~1/4 of
                # x has landed; ordered by first use, spread across queues ----
                xT_cp = xT_d.rearrange("(c p) n -> p c n", p=P)
                nc.sync.dma_start(out=xt[:, :, 0:512], in_=xT_cp[:, :, 0:512])
                nc.scalar.dma_start(out=wq, in_=wq_d.rearrange("(c p) f -> p c f", p=P))
                nc.scalar.dma_start(out=wk, in_=wk_d.rearrange("(c p) f -> p c f", p=P))
                nc.sync.dma_start(out=wv, in_=wv_d.rearrange("(c p) f -> p c f", p=P))
                for cc in range(1, NQ):
                    cs_ = slice(cc * 512, (cc + 1) * 512)
                    (nc.sync if cc % 2 else nc.scalar).dma_start(
                        out=xt[:, :, cs_], in_=xT_cp[:, :, cs_]
                    )
                nc.gpsimd.dma_start(out=wp, in_=wp_d.rearrange("(c p) f -> p c f", p=P))

                # ---- PE warm-up: ~4.5us of back-to-back K=1 matmuls on the
                # ones tile while the input DMAs land, so the p-state ramp
                # (0.65 -> 2.4 GHz after 3us of continuous execution) finishes
                # before the first real projection. Only for single-shot
                # builds: inside a For_i loop the PE comes out of the previous
                # iteration's projection tail already hot, and the warm-up
                # would just burn ~6us of PE time per iteration.
                if loop_n is None:
                    for _ in range(12):
                        wps = psum.tile([64, 512], f32, tag="sc", bufs=3, name="ps_warm")
                        nc.tensor.matmul(
                            wps, ones[64:65, :64], ones[64:65, :], start=True, stop=True
                        )

                def write_dummy_out(src_ap):
                    # consume `src_ap`-ish data so truncated variants aren't DCE'd
                    ncols = 1
                    for step, num in src_ap.ap[1:]:
                        ncols *= num
                    ncols = min(ncols, D)
                    for nt in range(NT):
                        osb = work.tile([P, D], f16, tag="osb", bufs=2, name="osb")
                        nc.vector.tensor_copy(osb[:, :ncols], src_ap)
                        nc.sync.dma_start(out=out_d[nt * P : (nt + 1) * P, :], in_=osb)

                def emit_qk_chunk(m, di, n1c):
                    """one (dst, n1-chunk) of the q^T/k^T projection for pair m."""
                    dst, w = ((qT, wq), (kT, wk))[di]
                    ps = psum.tile([P, 512], f32, tag="sc", bufs=3, name="ps_qk")
                    for kc in range(KC):
                        nc.tensor.matmul(
                            ps,
                            w[:, kc, m * P : (m + 1) * P],
                            xt[:, kc, n1c * 512 : (n1c + 1) * 512],
                            start=(kc == 0),
                            stop=(kc == KC - 1),
                        )
                    nc.vector.tensor_copy(dst[:, m, n1c * 512 : (n1c + 1) * 512], ps)

                def emit_qk(m):
                    for n1c in range(NQ):
                        for di in range(2):
                            emit_qk_chunk(m, di, n1c)

                def emit_v(nt):
                    """v projection for sequence tile nt -> vext f16 (all heads)."""
                    psv = psum.tile([P, HCOLS], f32, tag="sc", bufs=3, name="ps_v")
                    for kc in range(KC):
                        nc.tensor.matmul(
                            psv[:, :HCOLS],
                            xt[:, kc, nt * P : (nt + 1) * P],
                            wv[:, kc, :],
                            start=(kc == 0),
                            stop=(kc == KC - 1),
                        )
                    nc.vector.tensor_copy(
                        vext[:, :, nt, 0:64],
                        psv[:, :HCOLS].rearrange("p (h d) -> p h d", h=HEADS_PER_CORE),
                    )

                # eS^T tiles of completed (pair, half, t2-half) spans,
                # consumed by flipped attn@v passes scheduled into later slack
                psos = {}
                # pass-1 partial sums ([n1, 65] per head per n1-tile), SBUF
                parts = {}

                def emit_attnv1(hp, half, nt1):
                    """flipped attn@v pass 1 for n1-tile nt1: contract t2 0-7
                    from the half's low eS^T span; park partials in SBUF."""
                    eh = psos[(hp, half, 0)]
                    part = parts[(hp, half)]
                    for a in range(2):
                        pso = psum.tile([P, 65], f32, tag="po", bufs=2, name="pso")
                        for t2 in range(TP):
                            nc.tensor.matmul(
                                pso,
                                eh[:, a, t2, nt1 * P : (nt1 + 1) * P],
                                vext[:, 2 * hp + a, t2, 0:65],
                                start=(t2 == 0),
                                stop=(t2 == TP - 1),
                            )
                        nc.vector.tensor_copy(part[:, nt1, a, 0:65], pso)

                def emit_attnv2(hp, half, nt1, tail=False):
                    """pass 2: contract t2 8-15, add the parked partial, then
                    normalize per-partition (denominator is column 64) and
                    PE-transpose into ao[hp] (d-on-partitions for the proj).
                    In the tail the activation engine is idle, so the
                    normalize-scales run there instead of on DVE."""
                    eh = psos[(hp, half, 1)]
                    part = parts[(hp, half)]
                    aoN = work.tile([P, P], f16, tag="aoN", bufs=2, name="aoN")
                    for a in range(2):
                        pso = psum.tile([P, 65], f32, tag="po", bufs=2, name="pso")
                        for t2 in range(TP):
                            nc.tensor.matmul(
                                pso,
                                eh[:, a, t2, nt1 * P : (nt1 + 1) * P],
                                vext[:, 2 * hp + a, TP + t2, 0:65],
                                start=(t2 == 0),
                                stop=(t2 == TP - 1),
                            )
                        tot = work.tile([P, 65], f32, tag="tot", bufs=2, name="tot")
                        nc.vector.tensor_add(tot, pso, part[:, nt1, a, 0:65])
                        rcp = work.tile([P, 1], f32, tag="rcp", bufs=2, name="rcp")
                        nc.vector.reciprocal(rcp, tot[:, 64:65])
                        if tail:
                            nc.scalar.mul(aoN[:, a * 64 : (a + 1) * 64], tot[:, 0:64], rcp)
                        else:
                            nc.vector.tensor_scalar_mul(
                                aoN[:, a * 64 : (a + 1) * 64], tot[:, 0:64], rcp
                            )
                    pst = psum.tile([P, P], f16, tag="po", bufs=2, name="pst")
                    nc.tensor.transpose(pst, aoN, identity)
                    col = half * 1024 + nt1 * P
                    nc.vector.tensor_copy(ao[hp][:, col : col + P], pst)

                def emit_pair(hp, interleave=None):
                    """scores + exp for head pair hp; `interleave` maps
                    (half, t2) -> [fns] of PE work to emit inside the loop.
                    attn@v is NOT emitted here: the caller schedules
                    emit_attnv1 (needs exps t2 0-7, i.e. slots 8+) and
                    emit_attnv2 (needs all exps, i.e. the next phase)."""
                    inter = interleave or {}
                    for half in range(2):
                        if (hp, half) not in parts:
                            parts[(hp, half)] = work.tile(
                                [P, TP, 2, 66], f32, tag="part", bufs=2, name="part"
                            )
                        for t2 in range(NT):
                            if t2 % TP == 0:
                                if (hp, half, t2 // TP) not in psos:
                                    psos[(hp, half, t2 // TP)] = work.tile(
                                        [P, 2, TP, 1024], f16, tag="eh", bufs=3, name="eh"
                                    )
                                eh = psos[(hp, half, t2 // TP)]
                            pssc = [
                                psum.tile([P, 1024], f32, tag="sc", bufs=3, name=f"pssc{a}")
                                for a in range(2)
                            ]
                            for q in range(2):
                                n1c = 2 * half + q
                                qs = slice(q * 512, (q + 1) * 512)
                                ns = slice(n1c * 512, (n1c + 1) * 512)
                                for a in range(2):
                                    nc.tensor.matmul(
                                        pssc[a][:, qs],
                                        kT[64 * a : 64 * a + 64, hp, t2 * P : (t2 + 1) * P],
                                        qT[64 * a : 64 * a + 64, hp, ns],
                                        start=True,
                                        stop=True,
                                        tile_position=(64 * a, 0),
                                    )
                            for a in range(2):
                                nc.scalar.activation(
                                    eh[:, a, t2 % TP, :],
                                    pssc[a],
                                    bass.mybir.ActivationFunctionType.Exp,
                                    bias=zbias,
                                    scale=SCALE,
                                )
                            for fn in inter.get((half, t2), ()):
                                fn()

                def emit_proj(nt, copy_eng=None):
                    """full projection for tile nt (both pairs accumulate in
                    PSUM), copy to SBUF, DMA out. Requires both divisions for
                    the columns of tile nt."""
                    pj = psum.tile([P, 1024], f32, tag="sc", bufs=3, name="ps_pj")
                    for jc in range(2):
                        for dk in range(2):
                            nc.tensor.matmul(
                                pj[:, jc * 512 : (jc + 1) * 512],
                                ao[dk][:, nt * P : (nt + 1) * P],
                                wp[:, dk, jc * 512 : (jc + 1) * 512],
                                start=(dk == 0),
                                stop=(dk == 1),
                            )
                    osb = work.tile([P, D], f16, tag="osb", bufs=3, name="osb")
                    if copy_eng is nc.scalar:
                        nc.scalar.copy(osb, pj)
                    else:
                        nc.vector.tensor_copy(osb, pj)
                    nc.sync.dma_start(out=out_d[nt * P : (nt + 1) * P, :], in_=osb)

                def _emit():
                    if stop_after == "dma":
                        write_dummy_out(xt[:, 0, :D])
                        return
                    if stop_after == "projonly":
                        for hp in range(2):
                            nc.vector.memset(ao[hp], 0.001)
                        for nt in range(NT):
                            emit_proj(nt)
                        return

                    # minimal prologue: the first scores of pair 0 need q
                    # chunks 0-1, k chunk 0 and v tiles 0-1
                    emit_qk_chunk(0, 0, 0)
                    emit_qk_chunk(0, 1, 0)
                    emit_qk_chunk(0, 0, 1)
                    emit_v(0)
                    emit_v(1)
                    if stop_after == "qkv":
                        for n1c in range(1, NQ):
                            emit_qk_chunk(0, 1, n1c)
                        for n1c in range(2, NQ):
                            emit_qk_chunk(0, 0, n1c)
                        for nt in range(2, NT):
                            emit_v(nt)
                        emit_qk(1)
                        write_dummy_out(qT[:, 0, :D])
                        write_dummy_out(kT[:, 1, :D])
                        return

                    # Software pipelining across For_i iterations (timing
                    # builds): the previous iteration's last attn@v pass and
                    # projections 6-15 run inside THIS iteration's score/exp
                    # slack. The eh/part tiles are pre-created so the
                    # prev-iteration references resolve to the same pool slots
                    # (reader-before-writer in program order = previous
                    # iteration's data; the pool's WAR tracking orders the
                    # overwrites). Single-shot builds keep the inline tail.
                    # (measured on HW: the FULL cross-iteration pipelining came
                    # out ~2% slower than the plain schedule — the moved tail
                    # work overloads pair-0 half-0's PE budget — so it stays
                    # off. The lighter projection-only variant below is used
                    # for loop builds instead.)
                    pipe = False and loop_n is not None and stop_after is None
                    # projection-only pipelining: the previous iteration's
                    # projections 6-15 ride in this iteration's slack. They
                    # read only the persistent ao tiles; each is emitted
                    # before the attnv2 chunk that overwrites its columns, so
                    # the pool's WAR tracking gives exact loop semantics.
                    ppipe = (
                        loop_n is not None
                        and stop_after is None
                        and not os.environ.get("KERNEL_NO_PPIPE")
                    )
                    if pipe:
                        for hp_ in range(2):
                            for half_ in range(2):
                                parts[(hp_, half_)] = work.tile(
                                    [P, TP, 2, 66], f32, tag="part", bufs=2,
                                    name=f"part{hp_}{half_}",
                                )
                                for sp_ in range(2):
                                    psos[(hp_, half_, sp_)] = work.tile(
                                        [P, 2, TP, 1024], f16, tag="eh", bufs=3,
                                        name=f"eh{hp_}{half_}{sp_}",
                                    )

                    # pair 0 half 0: remaining qk(0) chunks just-in-time
                    # (k chunk c needed at t2=4c; q chunks 2-3 by half 1),
                    # v(2..15) just-in-time, attn@v pass 1 in slots 8-15,
                    # prev iteration's last attn@v pass 2 in slots 0-7
                    inter0 = {(0, t2): [lambda nt=t2 + 1: emit_v(nt)] for t2 in range(1, 15)}
                    inter0[(0, 1)].insert(0, lambda: emit_qk_chunk(0, 1, 1))
                    inter0[(0, 5)].insert(0, lambda: emit_qk_chunk(0, 1, 2))
                    inter0[(0, 9)].insert(0, lambda: emit_qk_chunk(0, 1, 3))
                    inter0[(0, 11)].insert(0, lambda: emit_qk_chunk(0, 0, 2))
                    inter0[(0, 13)].insert(0, lambda: emit_qk_chunk(0, 0, 3))
                    qk1 = [(1, di, n1c) for n1c in range(NQ) for di in range(2)]
                    for nt1 in range(TP):
                        inter0.setdefault((0, 8 + nt1), []).append(
                            lambda a=nt1: emit_attnv1(0, 0, a)
                        )
                        inter0.setdefault((1, nt1), []).append(
                            lambda a=nt1: emit_attnv2(0, 0, a)
                        )
                        inter0.setdefault((1, 8 + nt1), []).append(
                            lambda a=nt1: emit_attnv1(0, 1, a)
                        )
                    if pipe:
                        for nt1 in range(TP):
                            inter0.setdefault((0, nt1), []).insert(
                                0, lambda a=nt1: emit_attnv2(1, 1, a)
                            )
                        # prev-iteration projections 6..15 (ao columns 768+:
                        # overwritten only 6+ slots later / in pair 1)
                        for i in range(10):
                            inter0.setdefault((1, i), []).append(
                                lambda a=6 + i: emit_proj(a)
                            )
                        for i, args in enumerate(qk1[:3]):
                            inter0.setdefault((1, 10 + 2 * i), []).append(
                                lambda a=args: emit_qk_chunk(*a)
                            )
                    else:
                        inter0.setdefault((0, 15), []).append(
                            lambda a=qk1[0]: emit_qk_chunk(*a)
                        )
                        for i, args in enumerate(qk1[1:4]):
                            inter0.setdefault((1, 2 * i + 1), []).append(
                                lambda a=args: emit_qk_chunk(*a)
                            )
                    if ppipe:
                        inter0.setdefault((1, 0), []).insert(0, lambda: emit_proj(6))
                        inter0.setdefault((1, 1), []).insert(0, lambda: emit_proj(7))
                    emit_pair(0, interleave=inter0)

                    # pair 1: finish pair 0's attn@v early in half 0; its own
                    # pass 1/2 chunks just-in-time; projections of the first
                    # row-half in half 1 once their ao columns are complete
                    inter1 = {}
                    if not pipe:
                        # qk(1) chunks 4-7 ride in pair-1 half-0's slack,
                        # just-in-time for their first reader
                        for i, args in enumerate(qk1[4:]):
                            inter1.setdefault((0, 1 + 2 * i), []).append(
                                lambda a=args: emit_qk_chunk(*a)
                            )
                    for nt1 in range(TP):
                        if ppipe:
                            inter1.setdefault((0, nt1), []).append(
                                lambda a=8 + nt1: emit_proj(a)
                            )
                        inter1.setdefault((0, nt1), []).append(
                            lambda a=nt1: emit_attnv2(0, 1, a)
                        )
                        inter1.setdefault((0, 8 + nt1), []).append(
                            lambda a=nt1: emit_attnv1(1, 0, a)
                        )
                        inter1.setdefault((1, nt1), []).append(
                            lambda a=nt1: emit_attnv2(1, 0, a)
                        )
                        inter1.setdefault((1, 8 + nt1), []).append(
                            lambda a=nt1: emit_attnv1(1, 1, a)
                        )
                        if nt1 < 6:
                            inter1.setdefault((1, 2 + nt1), []).append(
                                lambda a=nt1: emit_proj(a)
                            )
                    if pipe:
                        # rest of qk(1): k chunks c just-in-time for t2=4c,
                        # q chunks 2-3 before half 1
                        for i, args in enumerate(qk1[3:]):
                            inter1.setdefault((0, 1 + 2 * i), []).append(
                                lambda a=args: emit_qk_chunk(*a)
                            )
                    emit_pair(1, interleave=inter1)

                    if stop_after == "attn":
                        for nt1 in range(TP):
                            emit_attnv2(1, 1, nt1)
                        write_dummy_out(ao[0][:, :D])
                        write_dummy_out(ao[1][:, :D])
                        return

                    if ppipe:
                        # loop tail: just the last attn@v pass; its
                        # projections run in the NEXT iteration's slack
                        for nt1 in range(TP):
                            emit_attnv2(1, 1, nt1, tail=True)
                    elif not pipe:
                        # single-shot tail: pass 2 of the last half pipelined
                        # with the remaining projections; output copies
                        # alternate between DVE and the now-idle ACT engine
                        emit_proj(6)
                        emit_proj(7, copy_eng=nc.scalar)
                        for nt1 in range(TP):
                            emit_attnv2(1, 1, nt1, tail=True)
                            emit_proj(8 + nt1, copy_eng=(nc.scalar if nt1 % 2 else None))

                _emit()

    nc.finalize()
    return nc


def make_in_maps(x, w_qk, w_v, w_proj):
    """Slice + transpose + f16-cast full inputs into per-core input dicts."""
    in_maps = []
    xTb = [np.ascontiguousarray(x[b].T.astype(np.float16)) for b in range(B)]
    wqk16 = w_qk.astype(np.float16)
    wv16 = w_v.astype(np.float16)
    wp16 = w_proj.astype(np.float16)
    for c in range(N_CORES):
        b, g = divmod(c, N_CORES // B)
        h0 = g * HCOLS
        in_maps.append(
            {
                "xT": xTb[b],
                "wq": np.ascontiguousarray(wqk16[:, h0 : h0 + HCOLS]),
                "wk": np.ascontiguousarray(wqk16[:, D + h0 : D + h0 + HCOLS]),
                "wv": np.ascontiguousarray(wv16[:, h0 : h0 + HCOLS]),
                "wp": np.ascontiguousarray(wp16[h0 : h0 + HCOLS, :]),
            }
        )
    return in_maps


def combine_results(results, b_proj):
    gpb = N_CORES // B
    out = np.empty((B, N, D), dtype=np.float32)
    for b in range(B):
        acc = results[b * gpb]["outp"].astype(np.float32)
        for g in range(1, gpb):
            acc = acc + results[b * gpb + g]["outp"].astype(np.float32)
        out[b] = acc + b_proj[None, :]
    return out


_CACHE = {}


def _pjrt_runner(nc):
    """Build a sharded 8-core single-exec runner for `nc` (mimics
    bass2jax.run_bass_via_pjrt). Returns run_fn(in_maps) -> per-core out dicts,
    and timed_fn(in_maps, reps) -> best wall seconds for one execution."""
    import time

    import jax
    from jax.experimental.shard_map import shard_map
    from jax.sharding import Mesh, NamedSharding, PartitionSpec

    from concourse import bass2jax, mybir

    bass2jax.install_neuronx_cc_hook()

    # persistent compile cache: the harness's first kernel() call then skips
    # the multi-minute walrus compile when this program was built before
    try:
        jax.config.update("jax_compilation_cache_dir", "/tmp/jax_neff_cache")
        jax.config.update("jax_persistent_cache_min_compile_time_secs", 2.0)
    except Exception:
        pass

    partition_name = nc.partition_id_tensor.name if nc.partition_id_tensor else None

    in_names, out_names, out_avals, zero_outs = [], [], [], []
    for alloc in nc.m.functions[0].allocations:
        if not isinstance(alloc, mybir.MemoryLocationSet):
            continue
        name = alloc.memorylocations[0].name
        if alloc.kind == "ExternalInput":
            if name != partition_name:
                in_names.append(name)
        elif alloc.kind == "ExternalOutput":
            out_names.append(name)
            shape = tuple(alloc.tensor_shape)
            dtype = mybir.dt.np(alloc.dtype)
            out_avals.append(jax.core.ShapedArray(shape, dtype))
            zero_outs.append(np.zeros(shape, dtype))
    n_params = len(in_names)
    n_outs = len(out_names)
    all_names = in_names + out_names
    if partition_name is not None:
        all_names = all_names + [partition_name]

    def _body(*args):
        operands = list(args)
        if partition_name is not None:
            operands.append(bass2jax.partition_id_tensor())
        return tuple(
            bass2jax._bass_exec_p.bind(
                *operands,
                out_avals=tuple(out_avals),
                in_names=tuple(all_names),
                out_names=tuple(out_names),
                lowering_input_output_aliases=(),
                sim_require_finite=True,
                sim_require_nnan=True,
                nc=nc,
            )
        )

    devices = jax.devices()[:N_CORES]
    mesh = Mesh(np.asarray(devices), ("core",))
    spec = NamedSharding(mesh, PartitionSpec("core"))

    _shmapped = shard_map(
        _body,
        mesh=mesh,
        in_specs=(PartitionSpec("core"),) * (n_params + n_outs),
        out_specs=(PartitionSpec("core"),) * n_outs,
        check_rep=False,
    )
    fn = jax.jit(
        _shmapped,
        donate_argnums=tuple(range(n_params, n_params + n_outs)),
        keep_unused=True,
    )
    # timing variant: no donation, so the zero out-buffers stay valid and are
    # uploaded once (donated buffers would need a fresh 64MB upload per call)
    fn_nodonate = jax.jit(_shmapped, keep_unused=True)

    def _concat_inputs(in_maps):
        per_core = [[np.asarray(m[name]) for name in in_names] for m in in_maps]
        return [
            np.concatenate([per_core[c][i] for c in range(N_CORES)], axis=0)
            for i in range(n_params)
        ]

    def _zeros():
        return [
            jax.device_put(np.zeros((N_CORES * z.shape[0], *z.shape[1:]), z.dtype), spec)
            for z in zero_outs
        ]

    def run_fn(in_maps):
        ins = [jax.device_put(a, spec) for a in _concat_inputs(in_maps)]
        outs = fn(*ins, *_zeros())
        outs = [np.asarray(o) for o in outs]
        return [
            {
                name: outs[i].reshape(N_CORES, *out_avals[i].shape)[c]
                for i, name in enumerate(out_names)
            }
            for c in range(N_CORES)
        ]

    def timed_fn(in_maps, reps=7):
        ins = [jax.device_put(a, spec) for a in _concat_inputs(in_maps)]
        z = _zeros()
        o = fn_nodonate(*ins, *z)  # warm-up (compiles)
        jax.block_until_ready(o)
        best = float("inf")
        for _ in range(reps):
            t0 = time.perf_counter()
            o = fn_nodonate(*ins, *z)
            jax.block_until_ready(o)
            best = min(best, time.perf_counter() - t0)
        return best

    return run_fn, timed_fn


LOOP_A, LOOP_B = 32, 288


def measure_hw_time(in_maps, reps=30, stop_after=None):
    """Per-iteration HW time via wall-clock slope between two static loop
    counts (min-filtered over many reps to reject host jitter)."""
    fns = {}
    for ln in (LOOP_A, LOOP_B):
        key = ("loop_nc", ln, stop_after)
        if key not in _CACHE:
            _CACHE[key] = _pjrt_runner(
                build_program(loop_n=ln, stop_after=stop_after)
            )
        fns[ln] = _CACHE[key][1]
    times = {LOOP_A: float("inf"), LOOP_B: float("inf")}
    for _ in range(max(2, reps // 3)):
        for ln in (LOOP_A, LOOP_B):
            times[ln] = min(times[ln], fns[ln](in_maps, reps=3))
    per_iter = (times[LOOP_B] - times[LOOP_A]) / (LOOP_B - LOOP_A)
    return per_iter * 1e9, times


def get_runner():
    if "runner" not in _CACHE:
        _CACHE["runner"] = _pjrt_runner(build_program())
    return _CACHE["runner"]


def run_on_hw(x, w_qk, w_v, w_proj, b_proj):
    run_fn, _ = get_runner()
    in_maps = make_in_maps(x, w_qk, w_v, w_proj)
    results = run_fn(in_maps)
    return combine_results(results, b_proj)


def kernel(x, w_qk, w_v, w_proj, b_proj):
    x = np.asarray(x, dtype=np.float32)
    w_qk = np.asarray(w_qk, dtype=np.float32)
    w_v = np.asarray(w_v, dtype=np.float32)
    w_proj = np.asarray(w_proj, dtype=np.float32)
    b_proj = np.asarray(b_proj, dtype=np.float32)
    return run_on_hw(x, w_qk, w_v, w_proj, b_proj)
